# revision 12
# baseline (speedup 1.0000x reference)
"""Multi-head attention (B=2, S=2048, D=1024, H=16, dk=64) on 8 Trainium2 NeuronCores.

Sharding: core c = (batch b = c//4, head-group g = c%4); each core handles one
batch and 4 heads (256 of the 1024 projection columns).  Per core:
  qT/kT projections in transposed [col, token] layout, v in [token, col] layout,
  scores computed transposed (S^T[k, q]) so softmax-exp feeds straight into the
  P^T-consuming attn matmul; softmax denominator rides along as a ones column
  appended to v.  Output projection is row-parallel (Megatron): each core emits
  a partial [2048, 1024] which the host sums.  Matmuls run in fp32r (near-fp32
  precision at bf16 throughput).

Bias handling (exact algebra, not approximations):
  - k bias dropped: adds a per-query constant to scores -> softmax invariant.
  - v bias + o bias folded into a host-side additive constant bo + Wo @ bv.
  - q bias and the 1/sqrt(dk) scale folded into Wq/bq host-side.
"""

import numpy as np

import concourse.bacc as bacc
import concourse.mybir as mybir
import concourse.tile as tile
from concourse.bass_utils import run_bass_kernel_spmd

F32 = mybir.dt.float32
F32R = mybir.dt.float32r
EXP = mybir.ActivationFunctionType.Exp

B = 2          # batches
S = 2048       # sequence length
D = 1024       # d_model
DK = 64        # head dim
GROUPS = 4     # head-groups -> 8 cores = B * GROUPS
HG = 4         # heads per core
CC = HG * DK   # 256 projection columns per core
P = 128
KC = D // P    # 8 contraction chunks for projections
NQ5 = S // 512  # 4 q-chunks of 512
NKT = S // P    # 16 k-token chunks of 128
CT = CC // P    # 2 column-tiles per core

_CACHE = {}


def _build_nc():
    nc = bacc.Bacc("TRN2", target_bir_lowering=False, debug=False, num_devices=8)

    xq = nc.dram_tensor("xq", [D, S], F32R, kind="ExternalInput")
    xk = nc.dram_tensor("xk", [D, S], F32R, kind="ExternalInput")
    xv = nc.dram_tensor("xv", [D, S], F32R, kind="ExternalInput")
    wq = nc.dram_tensor("wq", [D, CC], F32R, kind="ExternalInput")
    wk = nc.dram_tensor("wk", [D, CC], F32R, kind="ExternalInput")
    wv = nc.dram_tensor("wv", [D, CC], F32R, kind="ExternalInput")
    wo = nc.dram_tensor("wo", [CC, D], F32R, kind="ExternalInput")
    bq = nc.dram_tensor("bq", [P, CT], F32, kind="ExternalInput")
    out = nc.dram_tensor("out", [S, D], F32, kind="ExternalOutput")

    xq_v = xq.ap().rearrange("(kc p) t -> p kc t", p=P)
    xk_v = xk.ap().rearrange("(kc p) t -> p kc t", p=P)
    xv_v = xv.ap().rearrange("(kc p) t -> p kc t", p=P)
    out_v = out.ap().rearrange("(t p) n -> p t n", p=P)

    with tile.TileContext(nc) as tc:
        with (
            tc.tile_pool(name="wpool", bufs=1) as wpool,
            tc.tile_pool(name="kv", bufs=4) as kv,
            tc.tile_pool(name="xin", bufs=4) as xin,
            tc.tile_pool(name="qt", bufs=3) as qt_pool,
            tc.tile_pool(name="pt", bufs=4) as pt_pool,
            tc.tile_pool(name="comb", bufs=3) as comb_pool,
            tc.tile_pool(name="outs", bufs=3) as outs_pool,
            tc.tile_pool(name="small", bufs=3) as small,
            tc.tile_pool(name="mm_ps", bufs=2, space="PSUM") as mm_ps,
            tc.tile_pool(name="st_ps", bufs=2, space="PSUM") as st_ps,
            tc.tile_pool(name="at_ps", bufs=2, space="PSUM") as at_ps,
        ):
            # ---- weights to SBUF ----
            wq_sb = wpool.tile([P, KC, CC], F32R, tag="wq")
            wk_sb = wpool.tile([P, KC, CC], F32R, tag="wk")
            wv_sb = wpool.tile([P, KC, CC], F32R, tag="wv")
            wo_sb = wpool.tile([P, CT, D], F32R, tag="wo")
            bq_sb = wpool.tile([P, CT], F32, tag="bq")
            nc.sync.dma_start(wk_sb[:], wk.ap().rearrange("(kc p) c -> p kc c", p=P))
            nc.sync.dma_start(wq_sb[:], wq.ap().rearrange("(kc p) c -> p kc c", p=P))
            nc.sync.dma_start(wv_sb[:], wv.ap().rearrange("(kc p) c -> p kc c", p=P))
            nc.sync.dma_start(bq_sb[:], bq.ap())

            # ---- persistent activations, split per 512-token range so attention
            # can start as soon as the first ranges are projected ----
            kT_t = [kv.tile([P, CT, 512], F32R, tag="kT", name=f"kT{i}") for i in range(NQ5)]
            v_t = [kv.tile([P, 4, HG * (DK + 1)], F32R, tag="v", name=f"v{i}") for i in range(NQ5)]
            for t5 in range(NQ5):
                for h in range(HG):
                    nc.vector.memset(v_t[t5][:, :, h * (DK + 1) + DK].bitcast(F32), 1.0)

            def emit_kproj(t5):
                xkt = xin.tile([P, KC, 512], F32R, tag="xin")
                for kc in range(KC):
                    nc.sync.dma_start(xkt[:, kc], xk_v[:, kc, t5 * 512:(t5 + 1) * 512])
                for ct in range(CT):
                    ps = mm_ps.tile([P, 512], F32, tag="mm")
                    for kc in range(KC):
                        nc.tensor.matmul(
                            ps[:], wk_sb[:, kc, ct * P:(ct + 1) * P], xkt[:, kc],
                            start=(kc == 0), stop=(kc == KC - 1),
                        )
                    nc.vector.tensor_copy(kT_t[t5][:, ct, :], ps[:])

            def emit_vproj(t5):
                xvt = xin.tile([P, KC, 512], F32R, tag="xin")
                for kc in range(KC):
                    nc.sync.dma_start(xvt[:, kc], xv_v[:, kc, t5 * 512:(t5 + 1) * 512])
                for tt in range(4):
                    ps = mm_ps.tile([P, 512], F32, tag="mm")
                    for kc in range(KC):
                        nc.tensor.matmul(
                            ps[:, :CC], xvt[:, kc, tt * P:(tt + 1) * P], wv_sb[:, kc],
                            start=(kc == 0), stop=(kc == KC - 1),
                        )
                    dst = v_t[t5][:, tt, :].rearrange("p (h u) -> p h u", u=DK + 1)[:, :, :DK]
                    src = ps[:, :CC].rearrange("p (h u) -> p h u", u=DK)
                    nc.vector.tensor_copy(dst, src)

            qts = [None] * NQ5

            def emit_qproj(q5):
                qs = slice(q5 * 512, (q5 + 1) * 512)
                xqt = xin.tile([P, KC, 512], F32R, tag="xin")
                for kc in range(KC):
                    nc.sync.dma_start(xqt[:, kc], xq_v[:, kc, qs])
                qt = qt_pool.tile([P, CT, 512], F32R, tag="qt")
                for ct in range(CT):
                    ps = mm_ps.tile([P, 512], F32, tag="mm")
                    for kc in range(KC):
                        nc.tensor.matmul(
                            ps[:], wq_sb[:, kc, ct * P:(ct + 1) * P], xqt[:, kc],
                            start=(kc == 0), stop=(kc == KC - 1),
                        )
                    nc.vector.tensor_scalar_add(qt[:, ct], ps[:], bq_sb[:, ct:ct + 1])
                qts[q5] = qt

            emit_kproj(0)
            emit_qproj(0)
            emit_vproj(0)
            nc.sync.dma_start(wo_sb[:], wo.ap().rearrange("(ct p) n -> p ct n", p=P))
            for t5 in range(1, NQ5):
                emit_kproj(t5)
                emit_vproj(t5)

            # ---- per q-chunk: attention (head pairs, row-tiled scores), out proj ----
            for q5 in range(NQ5):
                if q5 + 1 < NQ5:
                    emit_qproj(q5 + 1)
                qt = qts[q5]
                comb = comb_pool.tile([P, CT, 512], F32R, tag="comb")
                for hp in range(CT):  # head pair = (2*hp, 2*hp+1)
                    ats = [at_ps.tile([P, 512], F32, tag="at", name=f"at{j}") for j in range(2)]
                    for kc in range(NKT):
                        st = st_ps.tile([P, 2, 512], F32, tag="st")
                        for j in range(2):
                            r = 64 * j
                            nc.tensor.matmul(
                                st[:, j],
                                kT_t[kc // 4][r:r + DK, hp, (kc % 4) * P:(kc % 4 + 1) * P],
                                qt[r:r + DK, hp],
                                start=True, stop=True,
                            )
                        ptt = pt_pool.tile([P, 2, 512], F32R, tag="pt")
                        nc.scalar.activation(ptt[:], st[:], EXP)
                        for j in range(2):
                            h = 2 * hp + j
                            nc.tensor.matmul(
                                ats[j][:DK + 1],
                                v_t[kc // 4][:, kc % 4, h * (DK + 1):(h + 1) * (DK + 1)],
                                ptt[:, j],
                                start=(kc == 0), stop=(kc == NKT - 1),
                            )
                    # normalize: rows 0..63 = unnormalized attn^T, row 64 = denom
                    for j in range(2):
                        at = ats[j]
                        rc = small.tile([1, 512], F32R, tag="rc")
                        with nc.allow_low_precision(reason="softmax reciprocal in fp32r"):
                            nc.vector.reciprocal(rc[:], at[DK:DK + 1])
                        rcb = small.tile([DK, 512], F32R, tag="rcb")
                        nc.gpsimd.partition_broadcast(rcb[:], rc[:])
                        nc.vector.tensor_mul(comb[64 * j:64 * (j + 1), hp], at[:DK], rcb[:])

                # ---- output projection for these 512 tokens ----
                for tt in range(4):
                    t128 = slice(tt * P, (tt + 1) * P)
                    out_t = outs_pool.tile([P, D], F32, tag="out")
                    for n2 in range(2):
                        ps = mm_ps.tile([P, 512], F32, tag="mm")
                        for c2 in range(CT):
                            nc.tensor.matmul(
                                ps[:], comb[:, c2, t128], wo_sb[:, c2, n2 * 512:(n2 + 1) * 512],
                                start=(c2 == 0), stop=(c2 == CT - 1),
                            )
                        nc.vector.tensor_copy(out_t[:, n2 * 512:(n2 + 1) * 512], ps[:])
                    nc.sync.dma_start(out_v[:, q5 * 4 + tt, :], out_t[:])

    nc.compile()
    return nc


def _get_nc():
    if "nc" not in _CACHE:
        _CACHE["nc"] = _build_nc()
    return _CACHE["nc"]


def kernel(query, key, value, Wq, bq, Wk, bk, Wv, bv, Wo, bo):
    nc = _get_nc()
    scale = np.float32(1.0 / np.sqrt(DK))

    query = np.asarray(query, dtype=np.float32)
    key = np.asarray(key, dtype=np.float32)
    value = np.asarray(value, dtype=np.float32)
    Wq = np.asarray(Wq, dtype=np.float32)
    Wk = np.asarray(Wk, dtype=np.float32)
    Wv = np.asarray(Wv, dtype=np.float32)
    Wo = np.asarray(Wo, dtype=np.float32)

    xq_np = [np.ascontiguousarray(query[b].T) for b in range(B)]
    xk_np = [np.ascontiguousarray(key[b].T) for b in range(B)]
    xv_np = [np.ascontiguousarray(value[b].T) for b in range(B)]

    wq_np, wk_np, wv_np, wo_np, bq_np = [], [], [], [], []
    for g in range(GROUPS):
        gsl = slice(CC * g, CC * (g + 1))
        wq_np.append(np.ascontiguousarray((Wq[gsl] * scale).T))
        wk_np.append(np.ascontiguousarray(Wk[gsl].T))
        wv_np.append(np.ascontiguousarray(Wv[gsl].T))
        wo_np.append(np.ascontiguousarray(Wo[:, gsl].T))
        bq_np.append(np.ascontiguousarray((np.asarray(bq, np.float32)[gsl] * scale).reshape(CT, P).T))

    in_maps = []
    for c in range(8):
        b, g = c // GROUPS, c % GROUPS
        in_maps.append({
            "xq": xq_np[b], "xk": xk_np[b], "xv": xv_np[b],
            "wq": wq_np[g], "wk": wk_np[g], "wv": wv_np[g],
            "wo": wo_np[g], "bq": bq_np[g],
        })

    res = run_bass_kernel_spmd(nc, in_maps, core_ids=list(range(8)))

    # host combine: sum the 4 head-group partials per batch, add folded bias
    bias = (np.asarray(bo, np.float64)
            + np.asarray(Wo, np.float64) @ np.asarray(bv, np.float64)).astype(np.float32)
    out = np.empty((B, S, D), dtype=np.float32)
    for b in range(B):
        acc = res.results[b * GROUPS + 0]["out"].astype(np.float32)
        for g in range(1, GROUPS):
            acc = acc + res.results[b * GROUPS + g]["out"]
        out[b] = acc + bias
    return out
